# revision 44
# baseline (speedup 1.0000x reference)
"""Trainium2 Bass kernel for nn_FNS (spectral neural solver).

Pipeline per sample (grid G=256, N1=127, CH=16 complex channels):
  rsym = odd-extend(r); r_hat = fftshift(ifft2(rsym))     [purely real]
  h = cconv3(cconv2(cconv1(r_hat)))                        [3x3 SAME convs]
  h = h * wtheta                                           [complex pixelwise]
  y = cconv_adj1(cconv_adj2(cconv_adj3(h)))
  e = fft2(ifftshift(y)).real[:127, :127]

Device mapping (8 cores, data-parallel over batch, 4 samples/core):
  - DFT stages as matmuls against sine/cosine matrices (odd symmetry makes
    r_hat real: r_hat = -(4/G^2) * SshT.T @ r @ SshT).
  - Complex convs as real matmuls: activations [128 part = 4 row-groups x
    32 real channels, 66 rows (1-row halo), 256 cols]; per 2-row output
    tile, 9 (dy,dx) matmuls per group with K=32, M=32 bf16, accumulating
    in PSUM (dy via rhs row offset, dx via rhs col offset). Tap-major
    issue order runs the 4 groups on disjoint 32x32 PE tiles
    concurrently; a 64-aligned band->column rotation per t-step lets two
    t-steps overlap on 8 tiles. PSUM eviction (fp32->bf16 + rotate-back)
    alternates between ScalarE and DVE.
  - Spectral multiply: swapped-half copies via SBUF-SBUF DMA on the
    gpsimd queue, then 3 bf16 DVE ops per 4-row chunk with streamed
    bf16 wtheta tiles (sync queue).
  - Samples software-pipelined over parity-paired grid buffers:
    head(s+1) (DFT + forward convs, PE-heavy) issues before spectral(s)
    (DVE/DMA-only) so the spectral stage hides under the next sample's
    conv matmuls.
"""

import os
import sys

for _p in ("/opt/trn_rl_repo", "/root/.axon_site/_ro/trn_rl_repo"):
    if os.path.isdir(_p) and _p not in sys.path:
        sys.path.insert(0, _p)

import numpy as np

from concourse import bacc, tile, mybir
from concourse.bass_utils import run_bass_kernel_spmd

G = 256
N1 = 127
NS = 4          # samples per core
NCORES = 8
F32 = mybir.dt.float32
F32R = mybir.dt.float32r
BF16 = mybir.dt.bfloat16
SCALE = -4.0 / (G * G)


def _build_nc(ns=NS):
    nc = bacc.Bacc(None)
    r4 = nc.declare_dram_parameter("r4", [ns, N1, N1], F32, isOutput=False)
    wl = nc.declare_dram_parameter("wl", [ns, 128, 6, 9, 32], F32, isOutput=False)
    ta4 = nc.declare_dram_parameter("ta4", [ns, 128, 64, G], BF16, isOutput=False)
    tb4 = nc.declare_dram_parameter("tb4", [ns, 128, 64, G], BF16, isOutput=False)
    sst = nc.declare_dram_parameter("sst", [N1, G], F32, isOutput=False)
    cpt = nc.declare_dram_parameter("cpt", [128, 2, N1], F32, isOutput=False)
    spt = nc.declare_dram_parameter("spt", [128, 2, N1], F32, isOutput=False)
    spnt = nc.declare_dram_parameter("spnt", [128, 2, N1], F32, isOutput=False)
    ident = nc.declare_dram_parameter("ident", [128, 128], F32, isOutput=False)
    eo = nc.declare_dram_parameter("eo", [ns, N1, N1], F32, isOutput=True)

    with tile.TileContext(nc) as tc:
        with (
            tc.tile_pool(name="grid", bufs=1) as grid_pool,
            tc.tile_pool(name="const", bufs=1) as const_pool,
            tc.tile_pool(name="wpool", bufs=2) as wpool,
            tc.tile_pool(name="io", bufs=2) as io_pool,
            tc.tile_pool(name="tmp", bufs=2) as tmp_pool,
            tc.tile_pool(name="stg", bufs=2) as stg_pool,
            tc.tile_pool(name="ps", bufs=4, space="PSUM") as ps_pool,
            tc.tile_pool(name="ps2", bufs=1, space="PSUM") as ps2_pool,
        ):
            # persistent grids: per-sample-parity ping/pong activation buffers
            A0 = grid_pool.tile([128, 66, G + 2], BF16, tag="A0")
            A1 = grid_pool.tile([128, 66, G + 2], BF16, tag="A1")
            B0 = grid_pool.tile([128, 66, G + 2], BF16, tag="B0")
            B1 = grid_pool.tile([128, 66, G + 2], BF16, tag="B1")
            As = [A0, A1]
            Bs = [B0, B1]
            # full zero-init: pads must be 0 and stale bits can decode to
            # NaN (NaN x 0 = NaN through layer-0's zero-weight blocks).
            # gpsimd memset has a ~68us drain; use DVE + scalar instead.
            nc.vector.memset(A0[:], 0.0)
            nc.vector.memset(B0[:], 0.0)
            nc.scalar.memzero(A1[:])
            nc.scalar.memzero(B1[:])

            sst_sb = const_pool.tile([N1, G], F32)
            nc.sync.dma_start(sst_sb[:], sst[:])
            cpt_sb = const_pool.tile([128, 2, N1], F32)
            nc.sync.dma_start(cpt_sb[:], cpt[:])
            spt_sb = const_pool.tile([128, 2, N1], F32)
            nc.sync.dma_start(spt_sb[:], spt[:])
            spnt_sb = const_pool.tile([128, 2, N1], F32)
            nc.sync.dma_start(spnt_sb[:], spnt[:])
            id_sb = const_pool.tile([128, 128], F32)
            nc.sync.dma_start(id_sb[:], ident[:])
            cpt_bf = const_pool.tile([128, 2, N1], BF16)
            nc.vector.tensor_copy(cpt_bf[:], cpt_sb[:])
            spt_bf = const_pool.tile([128, 2, N1], BF16)
            nc.vector.tensor_copy(spt_bf[:], spt_sb[:])
            spnt_bf = const_pool.tile([128, 2, N1], BF16)
            nc.vector.tensor_copy(spnt_bf[:], spnt_sb[:])
            id_bf = const_pool.tile([128, 128], BF16)
            nc.vector.tensor_copy(id_bf[:], id_sb[:])

            def halo_fill(X):
                # row 0 of groups 1..3 <- row 64 of groups 0..2; row 65 of
                # groups 0..2 <- row 1 of groups 1..3. Grid-edge halos stay 0.
                # On ScalarE: these gate the next layer's first matmuls and
                # DVE ops queue behind spectral work in overlap windows.
                for g in range(1, 4):
                    nc.scalar.copy(X[32 * g:32 * g + 32, 0:1, :],
                                   X[32 * g - 32:32 * g, 64:65, :])
                for g in range(3):
                    nc.scalar.copy(X[32 * g:32 * g + 32, 65:66, :],
                                   X[32 * g + 32:32 * g + 64, 1:2, :])

            def conv_layer(src, dst, w_sb, layer):
                # dst rows 1..64, cols 1..256 (per group) = conv(src) via PSUM
                for t in range(32):
                    # rotate band->PE-column assignment per t so 2 t-steps in
                    # flight use 8 disjoint PE sub-tiles (not 4 diagonal ones);
                    # rotation is 64-partition-aligned so the rotate-back
                    # copies stay within legal engine partition windows
                    c = 2 * (t % 2)
                    P = ps_pool.tile([128, 2 * G], F32, tag="convps")
                    for n, (dy, dx) in enumerate(
                            (dy, dx) for dy in range(3) for dx in range(3)):
                        for i in range(4):
                            j = (i + c) % 4
                            lhsT = w_sb[32 * i:32 * i + 32, layer, 3 * dy + dx, :]
                            rhs = src[32 * i:32 * i + 32,
                                      2 * t + dy:2 * t + dy + 2, dx:dx + G]
                            nc.tensor.matmul(
                                P[32 * j:32 * j + 32, :],
                                lhsT,
                                rhs,
                                start=(n == 0),
                                stop=(n == 8),
                                tile_position=(32 * i, 32 * j),
                            )
                    Pr = P[:].rearrange("p (r x) -> p r x", r=2)
                    rows = slice(1 + 2 * t, 3 + 2 * t)
                    # all evictions on ScalarE: DVE evictions queue behind
                    # spectral ops in overlap windows and lag psum recycle
                    if c == 0:
                        nc.scalar.copy(dst[:, rows, 1:G + 1], Pr)
                    else:
                        nc.scalar.copy(dst[0:64, rows, 1:G + 1], Pr[64:128])
                        nc.scalar.copy(dst[64:128, rows, 1:G + 1], Pr[0:64])
                halo_fill(dst)

            def head(s):
                # weights + input DFT + forward convs for sample s
                A = As[s % 2]
                Bb = Bs[s % 2]
                # ---- weights for all 6 layers of this sample ----
                w_f = wpool.tile([128, 6, 9, 32], F32, tag="wf")
                nc.sync.dma_start(w_f[:], wl[s])
                w_sb = wpool.tile([128, 6, 9, 32], BF16, tag="w")
                nc.vector.tensor_copy(w_sb[:], w_f[:])

                # ---- stage A: r_hat = SCALE * SshT.T @ r @ SshT into A ch0 ----
                r_sb = stg_pool.tile([N1, N1], F32, tag="rin")
                nc.sync.dma_start(r_sb[:], r4[s])
                t2_sb = stg_pool.tile([128, 2, N1], F32, tag="t2")
                for c in range(2):
                    t2_ps = ps2_pool.tile([128, N1], F32, tag="sa")
                    nc.tensor.matmul(t2_ps[:], sst_sb[:, 128 * c:128 * (c + 1)],
                                     r_sb[:], start=True, stop=True)
                    nc.vector.tensor_copy(t2_sb[:, c, :], t2_ps[:])
                t2t_sb = stg_pool.tile([N1, 2, 128], F32, tag="t2t")
                for c in range(2):
                    tr_ps = ps2_pool.tile([N1, 128], F32, tag="sa")
                    nc.tensor.transpose(tr_ps[:], t2_sb[:, c, :], id_sb[:])
                    nc.vector.tensor_copy(t2t_sb[:, c, :], tr_ps[:])
                rhat_sb = stg_pool.tile([128, 2, G], BF16, tag="rhat")
                for c in range(2):
                    rh_ps = ps2_pool.tile([128, G], F32, tag="sa")
                    nc.tensor.matmul(rh_ps[:], t2t_sb[:, c, :], sst_sb[:],
                                     start=True, stop=True)
                    nc.vector.tensor_scalar_mul(rhat_sb[:, c, :], rh_ps[:], SCALE)
                for g in range(4):
                    h = 64 * (g % 2)
                    nc.sync.dma_start(A[32 * g:32 * g + 1, 1:65, 1:G + 1],
                                      rhat_sb[h:h + 64, g // 2, :])
                halo_fill(A)

                # ---- forward convs ----
                conv_layer(A, Bb, w_sb, 0)
                conv_layer(Bb, A, w_sb, 1)
                conv_layer(A, Bb, w_sb, 2)
                return w_sb

            def spectral(s):
                # ---- spectral multiply: A = Bb*Ta + swap16(Bb)*Tb ----
                A = As[s % 2]
                Bb = Bs[s % 2]
                for c in range(16):
                    r0 = 1 + 4 * c
                    ta_t = io_pool.tile([128, 4, G], BF16, tag="ta")
                    nc.sync.dma_start(ta_t[:], ta4[s, :, 4 * c:4 * c + 4, :])
                    tb_t = io_pool.tile([128, 4, G], BF16, tag="tb")
                    nc.sync.dma_start(tb_t[:], tb4[s, :, 4 * c:4 * c + 4, :])
                    hs_t = io_pool.tile([128, 4, G], BF16, tag="hs")
                    # half-swap DMAs on the scalar DGE queue (keeps the sync
                    # queue free for ta/tb loads and conv-coupled transfers)
                    for g in range(4):
                        p = 32 * g
                        nc.gpsimd.dma_start(hs_t[p:p + 16, :, :],
                                            Bb[p + 16:p + 32, r0:r0 + 4, 1:G + 1])
                        nc.gpsimd.dma_start(hs_t[p + 16:p + 32, :, :],
                                            Bb[p:p + 16, r0:r0 + 4, 1:G + 1])
                    m1 = tmp_pool.tile([128, 4, G], BF16, tag="m1")
                    nc.vector.tensor_mul(m1[:], Bb[:, r0:r0 + 4, 1:G + 1], ta_t[:])
                    m2 = tmp_pool.tile([128, 4, G], BF16, tag="m2")
                    nc.vector.tensor_mul(m2[:], hs_t[:], tb_t[:])
                    nc.vector.tensor_add(A[:, r0:r0 + 4, 1:G + 1], m1[:], m2[:])
                halo_fill(A)

            def tail(s, w_sb):
                # adjoint convs + output DFT for sample s
                A = As[s % 2]
                Bb = Bs[s % 2]
                conv_layer(A, Bb, w_sb, 3)
                conv_layer(Bb, A, w_sb, 4)
                conv_layer(A, Bb, w_sb, 5)

                # ---- stage Z: e = (C'y1r + S'y1i)C'^T + (C'y1i - S'y1r)S'^T ----
                y1r_k = stg_pool.tile([128, 2, G], BF16, tag="y1r")
                y1i_k = stg_pool.tile([128, 2, G], BF16, tag="y1i")
                for g in range(4):
                    h = 64 * (g % 2)
                    nc.sync.dma_start(y1r_k[h:h + 64, g // 2, :],
                                      Bb[32 * g:32 * g + 1, 1:65, 1:G + 1])
                    nc.sync.dma_start(y1i_k[h:h + 64, g // 2, :],
                                      Bb[32 * g + 16:32 * g + 17, 1:65, 1:G + 1])
                ur_ps = ps2_pool.tile([N1, G], F32, tag="u")
                ui_ps = ps2_pool.tile([N1, G], F32, tag="u2")
                for c in range(2):
                    nc.tensor.matmul(ur_ps[:], cpt_bf[:, c, :], y1r_k[:, c, :],
                                     start=(c == 0), stop=False)
                    nc.tensor.matmul(ur_ps[:], spt_bf[:, c, :], y1i_k[:, c, :],
                                     start=False, stop=(c == 1))
                    nc.tensor.matmul(ui_ps[:], cpt_bf[:, c, :], y1i_k[:, c, :],
                                     start=(c == 0), stop=False)
                    nc.tensor.matmul(ui_ps[:], spnt_bf[:, c, :], y1r_k[:, c, :],
                                     start=False, stop=(c == 1))
                ur_sb = stg_pool.tile([N1, G], BF16, tag="ur")
                ui_sb = stg_pool.tile([N1, G], BF16, tag="ui")
                nc.vector.tensor_copy(ur_sb[:], ur_ps[:])
                nc.vector.tensor_copy(ui_sb[:], ui_ps[:])
                urt_sb = stg_pool.tile([128, 2, N1], BF16, tag="urt")
                uit_sb = stg_pool.tile([128, 2, N1], BF16, tag="uit")
                for c in range(2):
                    tp = ps2_pool.tile([128, N1], BF16, tag="u")
                    nc.tensor.transpose(tp[:], ur_sb[:, 128 * c:128 * (c + 1)], id_bf[0:N1, 0:N1])
                    nc.vector.tensor_copy(urt_sb[:, c, :], tp[:])
                    tp2 = ps2_pool.tile([128, N1], BF16, tag="u2")
                    nc.tensor.transpose(tp2[:], ui_sb[:, 128 * c:128 * (c + 1)], id_bf[0:N1, 0:N1])
                    nc.vector.tensor_copy(uit_sb[:, c, :], tp2[:])
                e_ps = ps2_pool.tile([N1, N1], F32, tag="u")
                for c in range(2):
                    nc.tensor.matmul(e_ps[:], urt_sb[:, c, :], cpt_bf[:, c, :],
                                     start=(c == 0), stop=False)
                    nc.tensor.matmul(e_ps[:], uit_sb[:, c, :], spt_bf[:, c, :],
                                     start=False, stop=(c == 1))
                e_sb = stg_pool.tile([N1, N1], F32, tag="e")
                nc.vector.tensor_copy(e_sb[:], e_ps[:])
                nc.sync.dma_start(eo[s], e_sb[:])

            # software pipeline: spectral(s) overlaps head(s+1)'s PE work;
            # head(s+1) issues first so its DVE copies aren't stuck behind
            # spectral's conv3-gated ops in the DVE FIFO.
            w_live = {0: head(0)}
            for s in range(ns):
                if s + 1 < ns:
                    w_live[s + 1] = head(s + 1)
                spectral(s)
                tail(s, w_live.pop(s))

    nc.compile()
    return nc


def _pack_lhsT(Wc):
    # Wc: [co, ci, 3, 3] complex -> [9, 32, 32] real lhsT blocks
    co, ci = Wc.shape[0], Wc.shape[1]
    out = np.zeros((9, 32, 32), np.float32)
    for dy in range(3):
        for dx in range(3):
            w = Wc[:, :, dy, dx]
            blk = out[3 * dy + dx]
            blk[0:ci, 0:co] = w.real.T
            blk[16:16 + ci, 0:co] = -w.imag.T
            blk[0:ci, 16:16 + co] = w.imag.T
            blk[16:16 + ci, 16:16 + co] = w.real.T
    return out


def _consts():
    m = np.arange(N1)
    k = np.arange(G)
    sst = (((-1.0) ** (m + 1))[:, None]
           * np.sin(2 * np.pi * np.outer(m + 1, k) / G)).astype(np.float32)
    Cp = (((-1.0) ** m)[:, None] * np.cos(2 * np.pi * np.outer(m, k) / G))
    Sp = (((-1.0) ** m)[:, None] * np.sin(2 * np.pi * np.outer(m, k) / G))
    # cpt[p, c, m] = Cp[m, c*128+p]
    cpt = np.ascontiguousarray(
        Cp.T.reshape(2, 128, N1).transpose(1, 0, 2)).astype(np.float32)
    spt = np.ascontiguousarray(
        Sp.T.reshape(2, 128, N1).transpose(1, 0, 2)).astype(np.float32)
    return {
        "sst": sst,
        "cpt": cpt,
        "spt": spt,
        "spnt": -spt,
        "ident": np.eye(128, dtype=np.float32),
    }


def make_core_inputs(r, w1r, w1i, w2r, w2i, w3r, w3i, wtr, wti, ns=NS):
    """Build one core's input map from its batch shard (leading dim ns)."""
    W1 = w1r + 1j * w1i
    W2 = w2r + 1j * w2i
    W3 = w3r + 1j * w3i

    def adj(W):
        return np.conj(np.swapaxes(np.swapaxes(W, 1, 2), -2, -1))

    A1, A2, A3 = adj(W1), adj(W2), adj(W3)
    wl = np.zeros((ns, 128, 6, 9, 32), np.float32)
    for s in range(ns):
        for l, W in enumerate((W1[s], W2[s], W3[s], A3[s], A2[s], A1[s])):
            blk = _pack_lhsT(W)  # [9, 32, 32]
            if l == 0:
                # layer-1 input is purely real (r_hat); its imag channels
                # hold stale data from the previous sample - never read them
                blk[:, 16:32, :] = 0.0
            wl[s, :, l] = np.tile(blk.transpose(1, 0, 2), (4, 1, 1))
    ta = np.empty((ns, 4, 32, 64, G), np.float32)
    tb = np.empty((ns, 4, 32, 64, G), np.float32)
    for g in range(4):
        rows = slice(64 * g, 64 * g + 64)
        ta[:, g, 0:16] = wtr[:, :, rows, :]
        tb[:, g, 0:16] = -wti[:, :, rows, :]
        ta[:, g, 16:32] = wtr[:, :, rows, :]
        tb[:, g, 16:32] = wti[:, :, rows, :]
    import ml_dtypes
    out = {
        "r4": np.ascontiguousarray(r[:, 0]).astype(np.float32),
        "wl": wl,
        "ta4": ta.reshape(ns, 128, 64, G).astype(ml_dtypes.bfloat16),
        "tb4": tb.reshape(ns, 128, 64, G).astype(ml_dtypes.bfloat16),
    }
    out.update(_consts())
    return out


_NC_CACHE = {}


def kernel(r, w1r, w1i, w2r, w2i, w3r, w3i, wtr, wti):
    B = r.shape[0]
    ns = B // NCORES
    if ns not in _NC_CACHE:
        _NC_CACHE[ns] = _build_nc(ns)
    nc = _NC_CACHE[ns]
    in_maps = []
    for c in range(NCORES):
        sl = slice(c * ns, (c + 1) * ns)
        in_maps.append(make_core_inputs(
            np.asarray(r[sl]), np.asarray(w1r[sl]), np.asarray(w1i[sl]),
            np.asarray(w2r[sl]), np.asarray(w2i[sl]), np.asarray(w3r[sl]),
            np.asarray(w3i[sl]), np.asarray(wtr[sl]), np.asarray(wti[sl]),
            ns=ns))
    res = run_bass_kernel_spmd(nc, in_maps, list(range(NCORES)))
    outs = [res.results[i]["eo"] for i in range(NCORES)]
    e = np.concatenate(outs, axis=0).reshape(B, 1, N1, N1).astype(np.float32)
    return e



# revision 46
# speedup vs baseline: 1.1650x; 1.1650x over previous
"""Trainium2 Bass kernel for nn_FNS (spectral neural solver).

Pipeline per sample (grid G=256, N1=127, CH=16 complex channels):
  rsym = odd-extend(r); r_hat = fftshift(ifft2(rsym))     [purely real]
  h = cconv3(cconv2(cconv1(r_hat)))                        [3x3 SAME convs]
  h = h * wtheta                                           [complex pixelwise]
  y = cconv_adj1(cconv_adj2(cconv_adj3(h)))
  e = fft2(ifftshift(y)).real[:127, :127]

Device mapping (8 cores, data-parallel over batch, 4 samples/core):
  - DFT stages as matmuls against sine/cosine matrices (odd symmetry makes
    r_hat real: r_hat = -(4/G^2) * SshT.T @ r @ SshT).
  - Complex convs as real matmuls: activations [128 part = 4 row-groups x
    32 real channels, 66 rows (1-row halo), 256 cols]; per 2-row output
    tile, 9 (dy,dx) matmuls per group with K=32, M=32 bf16, accumulating
    in PSUM (dy via rhs row offset, dx via rhs col offset). Tap-major
    issue order runs the 4 groups on disjoint 32x32 PE tiles
    concurrently; a 64-aligned band->column rotation per t-step lets two
    t-steps overlap on 8 tiles. PSUM eviction (fp32->bf16 + rotate-back)
    alternates between ScalarE and DVE.
  - Spectral multiply: swapped-half copies via SBUF-SBUF DMA on the
    gpsimd queue, then 3 bf16 DVE ops per 4-row chunk with streamed
    bf16 wtheta tiles (sync queue).
  - Samples software-pipelined over parity-paired grid buffers:
    head(s+1) (DFT + forward convs, PE-heavy) issues before spectral(s)
    (DVE/DMA-only) so the spectral stage hides under the next sample's
    conv matmuls.
"""

import os
import sys

for _p in ("/opt/trn_rl_repo", "/root/.axon_site/_ro/trn_rl_repo"):
    if os.path.isdir(_p) and _p not in sys.path:
        sys.path.insert(0, _p)

import numpy as np

from concourse import bacc, tile, mybir
from concourse.bass_utils import run_bass_kernel_spmd

G = 256
N1 = 127
NS = 4          # samples per core
NCORES = 8
F32 = mybir.dt.float32
F32R = mybir.dt.float32r
BF16 = mybir.dt.bfloat16
SCALE = -4.0 / (G * G)


def _build_nc(ns=NS):
    nc = bacc.Bacc(None)
    r4 = nc.declare_dram_parameter("r4", [ns, N1, N1], F32, isOutput=False)
    wl = nc.declare_dram_parameter("wl", [ns, 128, 6, 9, 32], F32, isOutput=False)
    ta4 = nc.declare_dram_parameter("ta4", [ns, 128, 64, G], BF16, isOutput=False)
    tb4 = nc.declare_dram_parameter("tb4", [ns, 128, 64, G], BF16, isOutput=False)
    sst = nc.declare_dram_parameter("sst", [N1, G], F32, isOutput=False)
    cpt = nc.declare_dram_parameter("cpt", [128, 2, N1], F32, isOutput=False)
    spt = nc.declare_dram_parameter("spt", [128, 2, N1], F32, isOutput=False)
    spnt = nc.declare_dram_parameter("spnt", [128, 2, N1], F32, isOutput=False)
    ident = nc.declare_dram_parameter("ident", [128, 128], F32, isOutput=False)
    eo = nc.declare_dram_parameter("eo", [ns, N1, N1], F32, isOutput=True)

    with tile.TileContext(nc) as tc:
        with (
            tc.tile_pool(name="grid", bufs=1) as grid_pool,
            tc.tile_pool(name="const", bufs=1) as const_pool,
            tc.tile_pool(name="wpool", bufs=2) as wpool,
            tc.tile_pool(name="io", bufs=2) as io_pool,
            tc.tile_pool(name="tmp", bufs=2) as tmp_pool,
            tc.tile_pool(name="stg", bufs=2) as stg_pool,
            tc.tile_pool(name="ps", bufs=5, space="PSUM") as ps_pool,
            tc.tile_pool(name="ps2", bufs=1, space="PSUM") as ps2_pool,
        ):
            # persistent grids: per-sample-parity ping/pong activation buffers
            A0 = grid_pool.tile([128, 66, G + 2], BF16, tag="A0")
            A1 = grid_pool.tile([128, 66, G + 2], BF16, tag="A1")
            B0 = grid_pool.tile([128, 66, G + 2], BF16, tag="B0")
            B1 = grid_pool.tile([128, 66, G + 2], BF16, tag="B1")
            As = [A0, A1]
            Bs = [B0, B1]
            # full zero-init: pads must be 0 and stale bits can decode to
            # NaN (NaN x 0 = NaN through layer-0's zero-weight blocks).
            # gpsimd memset has a ~68us drain; use DVE + scalar instead.
            nc.vector.memset(A0[:], 0.0)
            nc.vector.memset(B0[:], 0.0)
            nc.scalar.memzero(A1[:])
            nc.scalar.memzero(B1[:])

            sst_sb = const_pool.tile([N1, G], F32)
            nc.sync.dma_start(sst_sb[:], sst[:])
            cpt_sb = const_pool.tile([128, 2, N1], F32)
            nc.sync.dma_start(cpt_sb[:], cpt[:])
            spt_sb = const_pool.tile([128, 2, N1], F32)
            nc.sync.dma_start(spt_sb[:], spt[:])
            spnt_sb = const_pool.tile([128, 2, N1], F32)
            nc.sync.dma_start(spnt_sb[:], spnt[:])
            id_sb = const_pool.tile([128, 128], F32)
            nc.sync.dma_start(id_sb[:], ident[:])
            cpt_bf = const_pool.tile([128, 2, N1], BF16)
            nc.vector.tensor_copy(cpt_bf[:], cpt_sb[:])
            spt_bf = const_pool.tile([128, 2, N1], BF16)
            nc.vector.tensor_copy(spt_bf[:], spt_sb[:])
            spnt_bf = const_pool.tile([128, 2, N1], BF16)
            nc.vector.tensor_copy(spnt_bf[:], spnt_sb[:])
            id_bf = const_pool.tile([128, 128], BF16)
            nc.vector.tensor_copy(id_bf[:], id_sb[:])

            def halo_fill(X):
                # row 0 of groups 1..3 <- row 64 of groups 0..2; row 65 of
                # groups 0..2 <- row 1 of groups 1..3. Grid-edge halos stay 0.
                for g in range(1, 4):
                    nc.vector.tensor_copy(X[32 * g:32 * g + 32, 0:1, :],
                                          X[32 * g - 32:32 * g, 64:65, :])
                for g in range(3):
                    nc.vector.tensor_copy(X[32 * g:32 * g + 32, 65:66, :],
                                          X[32 * g + 32:32 * g + 64, 1:2, :])

            def conv_layer(src, dst, w_sb, layer):
                # dst rows 1..64, cols 1..256 (per group) = conv(src) via PSUM
                for t in range(32):
                    # rotate band->PE-column assignment per t so 2 t-steps in
                    # flight use 8 disjoint PE sub-tiles (not 4 diagonal ones);
                    # rotation is 64-partition-aligned so the rotate-back
                    # copies stay within legal engine partition windows
                    c = 2 * (t % 2)
                    P = ps_pool.tile([128, 2 * G], F32, tag="convps")
                    for n, (dy, dx) in enumerate(
                            (dy, dx) for dy in range(3) for dx in range(3)):
                        for i in range(4):
                            j = (i + c) % 4
                            lhsT = w_sb[32 * i:32 * i + 32, layer, 3 * dy + dx, :]
                            rhs = src[32 * i:32 * i + 32,
                                      2 * t + dy:2 * t + dy + 2, dx:dx + G]
                            nc.tensor.matmul(
                                P[32 * j:32 * j + 32, :],
                                lhsT,
                                rhs,
                                start=(n == 0),
                                stop=(n == 8),
                                tile_position=(32 * i, 32 * j),
                            )
                    Pr = P[:].rearrange("p (r x) -> p r x", r=2)
                    rows = slice(1 + 2 * t, 3 + 2 * t)
                    # all evictions on ScalarE: DVE evictions queue behind
                    # spectral ops in overlap windows and lag psum recycle
                    if c == 0:
                        nc.scalar.copy(dst[:, rows, 1:G + 1], Pr)
                    else:
                        nc.scalar.copy(dst[0:64, rows, 1:G + 1], Pr[64:128])
                        nc.scalar.copy(dst[64:128, rows, 1:G + 1], Pr[0:64])
                halo_fill(dst)

            def head(s):
                # weights + input DFT + forward convs for sample s
                A = As[s % 2]
                Bb = Bs[s % 2]
                # ---- weights for all 6 layers of this sample ----
                w_f = wpool.tile([128, 6, 9, 32], F32, tag="wf")
                nc.sync.dma_start(w_f[:], wl[s])
                w_sb = wpool.tile([128, 6, 9, 32], BF16, tag="w")
                nc.vector.tensor_copy(w_sb[:], w_f[:])

                # ---- stage A: r_hat = SCALE * SshT.T @ r @ SshT into A ch0 ----
                r_sb = stg_pool.tile([N1, N1], F32, tag="rin")
                nc.sync.dma_start(r_sb[:], r4[s])
                t2_sb = stg_pool.tile([128, 2, N1], F32, tag="t2")
                for c in range(2):
                    t2_ps = ps2_pool.tile([128, N1], F32, tag="sa")
                    nc.tensor.matmul(t2_ps[:], sst_sb[:, 128 * c:128 * (c + 1)],
                                     r_sb[:], start=True, stop=True)
                    nc.vector.tensor_copy(t2_sb[:, c, :], t2_ps[:])
                t2t_sb = stg_pool.tile([N1, 2, 128], F32, tag="t2t")
                for c in range(2):
                    tr_ps = ps2_pool.tile([N1, 128], F32, tag="sa")
                    nc.tensor.transpose(tr_ps[:], t2_sb[:, c, :], id_sb[:])
                    nc.vector.tensor_copy(t2t_sb[:, c, :], tr_ps[:])
                rhat_sb = stg_pool.tile([128, 2, G], BF16, tag="rhat")
                for c in range(2):
                    rh_ps = ps2_pool.tile([128, G], F32, tag="sa")
                    nc.tensor.matmul(rh_ps[:], t2t_sb[:, c, :], sst_sb[:],
                                     start=True, stop=True)
                    nc.vector.tensor_scalar_mul(rhat_sb[:, c, :], rh_ps[:], SCALE)
                for g in range(4):
                    h = 64 * (g % 2)
                    nc.sync.dma_start(A[32 * g:32 * g + 1, 1:65, 1:G + 1],
                                      rhat_sb[h:h + 64, g // 2, :])
                halo_fill(A)

                # ---- forward convs ----
                conv_layer(A, Bb, w_sb, 0)
                conv_layer(Bb, A, w_sb, 1)
                conv_layer(A, Bb, w_sb, 2)
                return w_sb

            def spectral(s):
                # ---- spectral multiply: A = Bb*Ta + swap16(Bb)*Tb ----
                A = As[s % 2]
                Bb = Bs[s % 2]
                for c in range(16):
                    r0 = 1 + 4 * c
                    ta_t = io_pool.tile([128, 4, G], BF16, tag="ta")
                    nc.sync.dma_start(ta_t[:], ta4[s, :, 4 * c:4 * c + 4, :])
                    tb_t = io_pool.tile([128, 4, G], BF16, tag="tb")
                    nc.sync.dma_start(tb_t[:], tb4[s, :, 4 * c:4 * c + 4, :])
                    hs_t = io_pool.tile([128, 4, G], BF16, tag="hs")
                    # half-swap DMAs on the scalar DGE queue (keeps the sync
                    # queue free for ta/tb loads and conv-coupled transfers)
                    for g in range(4):
                        p = 32 * g
                        nc.gpsimd.dma_start(hs_t[p:p + 16, :, :],
                                            Bb[p + 16:p + 32, r0:r0 + 4, 1:G + 1])
                        nc.gpsimd.dma_start(hs_t[p + 16:p + 32, :, :],
                                            Bb[p:p + 16, r0:r0 + 4, 1:G + 1])
                    m1 = tmp_pool.tile([128, 4, G], BF16, tag="m1")
                    nc.vector.tensor_mul(m1[:], Bb[:, r0:r0 + 4, 1:G + 1], ta_t[:])
                    m2 = tmp_pool.tile([128, 4, G], BF16, tag="m2")
                    nc.vector.tensor_mul(m2[:], hs_t[:], tb_t[:])
                    nc.vector.tensor_add(A[:, r0:r0 + 4, 1:G + 1], m1[:], m2[:])
                halo_fill(A)

            def tail(s, w_sb):
                # adjoint convs + output DFT for sample s
                A = As[s % 2]
                Bb = Bs[s % 2]
                conv_layer(A, Bb, w_sb, 3)
                conv_layer(Bb, A, w_sb, 4)
                conv_layer(A, Bb, w_sb, 5)

                # ---- stage Z: e = (C'y1r + S'y1i)C'^T + (C'y1i - S'y1r)S'^T ----
                y1r_k = stg_pool.tile([128, 2, G], BF16, tag="y1r")
                y1i_k = stg_pool.tile([128, 2, G], BF16, tag="y1i")
                for g in range(4):
                    h = 64 * (g % 2)
                    nc.sync.dma_start(y1r_k[h:h + 64, g // 2, :],
                                      Bb[32 * g:32 * g + 1, 1:65, 1:G + 1])
                    nc.sync.dma_start(y1i_k[h:h + 64, g // 2, :],
                                      Bb[32 * g + 16:32 * g + 17, 1:65, 1:G + 1])
                ur_ps = ps2_pool.tile([N1, G], F32, tag="u")
                ui_ps = ps2_pool.tile([N1, G], F32, tag="u2")
                for c in range(2):
                    nc.tensor.matmul(ur_ps[:], cpt_bf[:, c, :], y1r_k[:, c, :],
                                     start=(c == 0), stop=False)
                    nc.tensor.matmul(ur_ps[:], spt_bf[:, c, :], y1i_k[:, c, :],
                                     start=False, stop=(c == 1))
                    nc.tensor.matmul(ui_ps[:], cpt_bf[:, c, :], y1i_k[:, c, :],
                                     start=(c == 0), stop=False)
                    nc.tensor.matmul(ui_ps[:], spnt_bf[:, c, :], y1r_k[:, c, :],
                                     start=False, stop=(c == 1))
                ur_sb = stg_pool.tile([N1, G], BF16, tag="ur")
                ui_sb = stg_pool.tile([N1, G], BF16, tag="ui")
                nc.vector.tensor_copy(ur_sb[:], ur_ps[:])
                nc.vector.tensor_copy(ui_sb[:], ui_ps[:])
                urt_sb = stg_pool.tile([128, 2, N1], BF16, tag="urt")
                uit_sb = stg_pool.tile([128, 2, N1], BF16, tag="uit")
                for c in range(2):
                    tp = ps2_pool.tile([128, N1], BF16, tag="u")
                    nc.tensor.transpose(tp[:], ur_sb[:, 128 * c:128 * (c + 1)], id_bf[0:N1, 0:N1])
                    nc.vector.tensor_copy(urt_sb[:, c, :], tp[:])
                    tp2 = ps2_pool.tile([128, N1], BF16, tag="u2")
                    nc.tensor.transpose(tp2[:], ui_sb[:, 128 * c:128 * (c + 1)], id_bf[0:N1, 0:N1])
                    nc.vector.tensor_copy(uit_sb[:, c, :], tp2[:])
                e_ps = ps2_pool.tile([N1, N1], F32, tag="u")
                for c in range(2):
                    nc.tensor.matmul(e_ps[:], urt_sb[:, c, :], cpt_bf[:, c, :],
                                     start=(c == 0), stop=False)
                    nc.tensor.matmul(e_ps[:], uit_sb[:, c, :], spt_bf[:, c, :],
                                     start=False, stop=(c == 1))
                e_sb = stg_pool.tile([N1, N1], F32, tag="e")
                nc.vector.tensor_copy(e_sb[:], e_ps[:])
                nc.sync.dma_start(eo[s], e_sb[:])

            # software pipeline: spectral(s) overlaps head(s+1)'s PE work;
            # head(s+1) issues first so its DVE copies aren't stuck behind
            # spectral's conv3-gated ops in the DVE FIFO.
            w_live = {0: head(0)}
            for s in range(ns):
                if s + 1 < ns:
                    w_live[s + 1] = head(s + 1)
                spectral(s)
                tail(s, w_live.pop(s))

    nc.compile()
    return nc


def _pack_lhsT(Wc):
    # Wc: [co, ci, 3, 3] complex -> [9, 32, 32] real lhsT blocks
    co, ci = Wc.shape[0], Wc.shape[1]
    out = np.zeros((9, 32, 32), np.float32)
    for dy in range(3):
        for dx in range(3):
            w = Wc[:, :, dy, dx]
            blk = out[3 * dy + dx]
            blk[0:ci, 0:co] = w.real.T
            blk[16:16 + ci, 0:co] = -w.imag.T
            blk[0:ci, 16:16 + co] = w.imag.T
            blk[16:16 + ci, 16:16 + co] = w.real.T
    return out


def _consts():
    m = np.arange(N1)
    k = np.arange(G)
    sst = (((-1.0) ** (m + 1))[:, None]
           * np.sin(2 * np.pi * np.outer(m + 1, k) / G)).astype(np.float32)
    Cp = (((-1.0) ** m)[:, None] * np.cos(2 * np.pi * np.outer(m, k) / G))
    Sp = (((-1.0) ** m)[:, None] * np.sin(2 * np.pi * np.outer(m, k) / G))
    # cpt[p, c, m] = Cp[m, c*128+p]
    cpt = np.ascontiguousarray(
        Cp.T.reshape(2, 128, N1).transpose(1, 0, 2)).astype(np.float32)
    spt = np.ascontiguousarray(
        Sp.T.reshape(2, 128, N1).transpose(1, 0, 2)).astype(np.float32)
    return {
        "sst": sst,
        "cpt": cpt,
        "spt": spt,
        "spnt": -spt,
        "ident": np.eye(128, dtype=np.float32),
    }


def make_core_inputs(r, w1r, w1i, w2r, w2i, w3r, w3i, wtr, wti, ns=NS):
    """Build one core's input map from its batch shard (leading dim ns)."""
    W1 = w1r + 1j * w1i
    W2 = w2r + 1j * w2i
    W3 = w3r + 1j * w3i

    def adj(W):
        return np.conj(np.swapaxes(np.swapaxes(W, 1, 2), -2, -1))

    A1, A2, A3 = adj(W1), adj(W2), adj(W3)
    wl = np.zeros((ns, 128, 6, 9, 32), np.float32)
    for s in range(ns):
        for l, W in enumerate((W1[s], W2[s], W3[s], A3[s], A2[s], A1[s])):
            blk = _pack_lhsT(W)  # [9, 32, 32]
            if l == 0:
                # layer-1 input is purely real (r_hat); its imag channels
                # hold stale data from the previous sample - never read them
                blk[:, 16:32, :] = 0.0
            wl[s, :, l] = np.tile(blk.transpose(1, 0, 2), (4, 1, 1))
    ta = np.empty((ns, 4, 32, 64, G), np.float32)
    tb = np.empty((ns, 4, 32, 64, G), np.float32)
    for g in range(4):
        rows = slice(64 * g, 64 * g + 64)
        ta[:, g, 0:16] = wtr[:, :, rows, :]
        tb[:, g, 0:16] = -wti[:, :, rows, :]
        ta[:, g, 16:32] = wtr[:, :, rows, :]
        tb[:, g, 16:32] = wti[:, :, rows, :]
    import ml_dtypes
    out = {
        "r4": np.ascontiguousarray(r[:, 0]).astype(np.float32),
        "wl": wl,
        "ta4": ta.reshape(ns, 128, 64, G).astype(ml_dtypes.bfloat16),
        "tb4": tb.reshape(ns, 128, 64, G).astype(ml_dtypes.bfloat16),
    }
    out.update(_consts())
    return out


_NC_CACHE = {}


def kernel(r, w1r, w1i, w2r, w2i, w3r, w3i, wtr, wti):
    B = r.shape[0]
    ns = B // NCORES
    if ns not in _NC_CACHE:
        _NC_CACHE[ns] = _build_nc(ns)
    nc = _NC_CACHE[ns]
    in_maps = []
    for c in range(NCORES):
        sl = slice(c * ns, (c + 1) * ns)
        in_maps.append(make_core_inputs(
            np.asarray(r[sl]), np.asarray(w1r[sl]), np.asarray(w1i[sl]),
            np.asarray(w2r[sl]), np.asarray(w2i[sl]), np.asarray(w3r[sl]),
            np.asarray(w3i[sl]), np.asarray(wtr[sl]), np.asarray(wti[sl]),
            ns=ns))
    res = run_bass_kernel_spmd(nc, in_maps, list(range(NCORES)))
    outs = [res.results[i]["eo"] for i in range(NCORES)]
    e = np.concatenate(outs, axis=0).reshape(B, 1, N1, N1).astype(np.float32)
    return e



# revision 47
# speedup vs baseline: 1.1680x; 1.0026x over previous
"""Trainium2 Bass kernel for nn_FNS (spectral neural solver).

Pipeline per sample (grid G=256, N1=127, CH=16 complex channels):
  rsym = odd-extend(r); r_hat = fftshift(ifft2(rsym))     [purely real]
  h = cconv3(cconv2(cconv1(r_hat)))                        [3x3 SAME convs]
  h = h * wtheta                                           [complex pixelwise]
  y = cconv_adj1(cconv_adj2(cconv_adj3(h)))
  e = fft2(ifftshift(y)).real[:127, :127]

Device mapping (8 cores, data-parallel over batch, 4 samples/core):
  - DFT stages as matmuls against sine/cosine matrices (odd symmetry makes
    r_hat real: r_hat = -(4/G^2) * SshT.T @ r @ SshT).
  - Complex convs as real matmuls: activations [128 part = 4 row-groups x
    32 real channels, 66 rows (1-row halo), 256 cols]; per 2-row output
    tile, 9 (dy,dx) matmuls per group with K=32, M=32 bf16, accumulating
    in PSUM (dy via rhs row offset, dx via rhs col offset). Tap-major
    issue order runs the 4 groups on disjoint 32x32 PE tiles
    concurrently; a 64-aligned band->column rotation per t-step lets two
    t-steps overlap on 8 tiles. PSUM eviction (fp32->bf16 + rotate-back)
    alternates between ScalarE and DVE.
  - Spectral multiply: swapped-half copies via SBUF-SBUF DMA on the
    gpsimd queue, then 3 bf16 DVE ops per 4-row chunk with streamed
    bf16 wtheta tiles (sync queue).
  - Samples software-pipelined over parity-paired grid buffers:
    head(s+1) (DFT + forward convs, PE-heavy) issues before spectral(s)
    (DVE/DMA-only) so the spectral stage hides under the next sample's
    conv matmuls.
"""

import os
import sys

for _p in ("/opt/trn_rl_repo", "/root/.axon_site/_ro/trn_rl_repo"):
    if os.path.isdir(_p) and _p not in sys.path:
        sys.path.insert(0, _p)

import numpy as np

from concourse import bacc, tile, mybir
from concourse.bass_utils import run_bass_kernel_spmd

G = 256
N1 = 127
NS = 4          # samples per core
NCORES = 8
F32 = mybir.dt.float32
F32R = mybir.dt.float32r
BF16 = mybir.dt.bfloat16
SCALE = -4.0 / (G * G)


def _build_nc(ns=NS):
    nc = bacc.Bacc(None)
    r4 = nc.declare_dram_parameter("r4", [ns, N1, N1], F32, isOutput=False)
    wl = nc.declare_dram_parameter("wl", [ns, 128, 6, 9, 32], F32, isOutput=False)
    ta4 = nc.declare_dram_parameter("ta4", [ns, 128, 64, G], BF16, isOutput=False)
    tb4 = nc.declare_dram_parameter("tb4", [ns, 128, 64, G], BF16, isOutput=False)
    sst = nc.declare_dram_parameter("sst", [N1, G], F32, isOutput=False)
    cpt = nc.declare_dram_parameter("cpt", [128, 2, N1], F32, isOutput=False)
    spt = nc.declare_dram_parameter("spt", [128, 2, N1], F32, isOutput=False)
    spnt = nc.declare_dram_parameter("spnt", [128, 2, N1], F32, isOutput=False)
    ident = nc.declare_dram_parameter("ident", [128, 128], F32, isOutput=False)
    eo = nc.declare_dram_parameter("eo", [ns, N1, N1], F32, isOutput=True)

    with tile.TileContext(nc) as tc:
        with (
            tc.tile_pool(name="grid", bufs=1) as grid_pool,
            tc.tile_pool(name="const", bufs=1) as const_pool,
            tc.tile_pool(name="wpool", bufs=2) as wpool,
            tc.tile_pool(name="io", bufs=2) as io_pool,
            tc.tile_pool(name="tmp", bufs=2) as tmp_pool,
            tc.tile_pool(name="stg", bufs=2) as stg_pool,
            tc.tile_pool(name="ps", bufs=5, space="PSUM") as ps_pool,
            tc.tile_pool(name="ps2", bufs=1, space="PSUM") as ps2_pool,
        ):
            # persistent grids: per-sample-parity ping/pong activation buffers
            A0 = grid_pool.tile([128, 66, G + 2], BF16, tag="A0")
            A1 = grid_pool.tile([128, 66, G + 2], BF16, tag="A1")
            B0 = grid_pool.tile([128, 66, G + 2], BF16, tag="B0")
            B1 = grid_pool.tile([128, 66, G + 2], BF16, tag="B1")
            As = [A0, A1]
            Bs = [B0, B1]
            # full zero-init: pads must be 0 and stale bits can decode to
            # NaN (NaN x 0 = NaN through layer-0's zero-weight blocks).
            # gpsimd memset has a ~68us drain; use DVE + scalar instead.
            nc.vector.memset(A0[:], 0.0)
            nc.vector.memset(B0[:], 0.0)
            nc.scalar.memzero(A1[:])
            nc.scalar.memzero(B1[:])

            sst_sb = const_pool.tile([N1, G], F32)
            nc.sync.dma_start(sst_sb[:], sst[:])
            cpt_sb = const_pool.tile([128, 2, N1], F32)
            nc.sync.dma_start(cpt_sb[:], cpt[:])
            spt_sb = const_pool.tile([128, 2, N1], F32)
            nc.sync.dma_start(spt_sb[:], spt[:])
            spnt_sb = const_pool.tile([128, 2, N1], F32)
            nc.sync.dma_start(spnt_sb[:], spnt[:])
            id_sb = const_pool.tile([128, 128], F32)
            nc.sync.dma_start(id_sb[:], ident[:])
            cpt_bf = const_pool.tile([128, 2, N1], BF16)
            nc.vector.tensor_copy(cpt_bf[:], cpt_sb[:])
            spt_bf = const_pool.tile([128, 2, N1], BF16)
            nc.vector.tensor_copy(spt_bf[:], spt_sb[:])
            spnt_bf = const_pool.tile([128, 2, N1], BF16)
            nc.vector.tensor_copy(spnt_bf[:], spnt_sb[:])
            id_bf = const_pool.tile([128, 128], BF16)
            nc.vector.tensor_copy(id_bf[:], id_sb[:])

            def halo_fill(X):
                # row 0 of groups 1..3 <- row 64 of groups 0..2; row 65 of
                # groups 0..2 <- row 1 of groups 1..3. Grid-edge halos stay 0.
                for g in range(1, 4):
                    nc.vector.tensor_copy(X[32 * g:32 * g + 32, 0:1, :],
                                          X[32 * g - 32:32 * g, 64:65, :])
                for g in range(3):
                    nc.vector.tensor_copy(X[32 * g:32 * g + 32, 65:66, :],
                                          X[32 * g + 32:32 * g + 64, 1:2, :])

            def conv_layer(src, dst, w_sb, layer):
                # dst rows 1..64, cols 1..256 (per group) = conv(src) via PSUM
                for t in range(32):
                    # rotate band->PE-column assignment per t so 2 t-steps in
                    # flight use 8 disjoint PE sub-tiles (not 4 diagonal ones);
                    # rotation is 64-partition-aligned so the rotate-back
                    # copies stay within legal engine partition windows
                    c = 2 * (t % 2)
                    P = ps_pool.tile([128, 2 * G], F32, tag="convps")
                    for n, (dy, dx) in enumerate(
                            (dy, dx) for dy in range(3) for dx in range(3)):
                        for i in range(4):
                            j = (i + c) % 4
                            lhsT = w_sb[32 * i:32 * i + 32, layer, 3 * dy + dx, :]
                            rhs = src[32 * i:32 * i + 32,
                                      2 * t + dy:2 * t + dy + 2, dx:dx + G]
                            nc.tensor.matmul(
                                P[32 * j:32 * j + 32, :],
                                lhsT,
                                rhs,
                                start=(n == 0),
                                stop=(n == 8),
                                tile_position=(32 * i, 32 * j),
                            )
                    Pr = P[:].rearrange("p (r x) -> p r x", r=2)
                    rows = slice(1 + 2 * t, 3 + 2 * t)
                    # all evictions on ScalarE: DVE evictions queue behind
                    # spectral ops in overlap windows and lag psum recycle
                    if c == 0:
                        nc.scalar.copy(dst[:, rows, 1:G + 1], Pr)
                    else:
                        nc.scalar.copy(dst[0:64, rows, 1:G + 1], Pr[64:128])
                        nc.scalar.copy(dst[64:128, rows, 1:G + 1], Pr[0:64])
                halo_fill(dst)

            def head(s):
                # weights + input DFT + forward convs for sample s
                A = As[s % 2]
                Bb = Bs[s % 2]
                # ---- weights for all 6 layers of this sample ----
                w_f = wpool.tile([128, 6, 9, 32], F32, tag="wf")
                nc.gpsimd.dma_start(w_f[:], wl[s])
                w_sb = wpool.tile([128, 6, 9, 32], BF16, tag="w")
                nc.vector.tensor_copy(w_sb[:], w_f[:])

                # ---- stage A: r_hat = SCALE * SshT.T @ r @ SshT into A ch0 ----
                r_sb = stg_pool.tile([N1, N1], F32, tag="rin")
                nc.gpsimd.dma_start(r_sb[:], r4[s])
                t2_sb = stg_pool.tile([128, 2, N1], F32, tag="t2")
                for c in range(2):
                    t2_ps = ps2_pool.tile([128, N1], F32, tag="sa")
                    nc.tensor.matmul(t2_ps[:], sst_sb[:, 128 * c:128 * (c + 1)],
                                     r_sb[:], start=True, stop=True)
                    nc.vector.tensor_copy(t2_sb[:, c, :], t2_ps[:])
                t2t_sb = stg_pool.tile([N1, 2, 128], F32, tag="t2t")
                for c in range(2):
                    tr_ps = ps2_pool.tile([N1, 128], F32, tag="sa")
                    nc.tensor.transpose(tr_ps[:], t2_sb[:, c, :], id_sb[:])
                    nc.vector.tensor_copy(t2t_sb[:, c, :], tr_ps[:])
                rhat_sb = stg_pool.tile([128, 2, G], BF16, tag="rhat")
                for c in range(2):
                    rh_ps = ps2_pool.tile([128, G], F32, tag="sa")
                    nc.tensor.matmul(rh_ps[:], t2t_sb[:, c, :], sst_sb[:],
                                     start=True, stop=True)
                    nc.vector.tensor_scalar_mul(rhat_sb[:, c, :], rh_ps[:], SCALE)
                for g in range(4):
                    h = 64 * (g % 2)
                    nc.gpsimd.dma_start(A[32 * g:32 * g + 1, 1:65, 1:G + 1],
                                        rhat_sb[h:h + 64, g // 2, :])
                halo_fill(A)

                # ---- forward convs ----
                conv_layer(A, Bb, w_sb, 0)
                conv_layer(Bb, A, w_sb, 1)
                conv_layer(A, Bb, w_sb, 2)
                return w_sb

            def spectral(s):
                # ---- spectral multiply: A = Bb*Ta + swap16(Bb)*Tb ----
                A = As[s % 2]
                Bb = Bs[s % 2]
                for c in range(16):
                    r0 = 1 + 4 * c
                    ta_t = io_pool.tile([128, 4, G], BF16, tag="ta")
                    nc.sync.dma_start(ta_t[:], ta4[s, :, 4 * c:4 * c + 4, :])
                    tb_t = io_pool.tile([128, 4, G], BF16, tag="tb")
                    nc.sync.dma_start(tb_t[:], tb4[s, :, 4 * c:4 * c + 4, :])
                    hs_t = io_pool.tile([128, 4, G], BF16, tag="hs")
                    # half-swap DMAs on the scalar DGE queue (keeps the sync
                    # queue free for ta/tb loads and conv-coupled transfers)
                    for g in range(4):
                        p = 32 * g
                        nc.gpsimd.dma_start(hs_t[p:p + 16, :, :],
                                            Bb[p + 16:p + 32, r0:r0 + 4, 1:G + 1])
                        nc.gpsimd.dma_start(hs_t[p + 16:p + 32, :, :],
                                            Bb[p:p + 16, r0:r0 + 4, 1:G + 1])
                    m1 = tmp_pool.tile([128, 4, G], BF16, tag="m1")
                    nc.vector.tensor_mul(m1[:], Bb[:, r0:r0 + 4, 1:G + 1], ta_t[:])
                    m2 = tmp_pool.tile([128, 4, G], BF16, tag="m2")
                    nc.vector.tensor_mul(m2[:], hs_t[:], tb_t[:])
                    nc.vector.tensor_add(A[:, r0:r0 + 4, 1:G + 1], m1[:], m2[:])
                halo_fill(A)

            def tail(s, w_sb):
                # adjoint convs + output DFT for sample s
                A = As[s % 2]
                Bb = Bs[s % 2]
                conv_layer(A, Bb, w_sb, 3)
                conv_layer(Bb, A, w_sb, 4)
                conv_layer(A, Bb, w_sb, 5)

                # ---- stage Z: e = (C'y1r + S'y1i)C'^T + (C'y1i - S'y1r)S'^T ----
                y1r_k = stg_pool.tile([128, 2, G], BF16, tag="y1r")
                y1i_k = stg_pool.tile([128, 2, G], BF16, tag="y1i")
                for g in range(4):
                    h = 64 * (g % 2)
                    nc.sync.dma_start(y1r_k[h:h + 64, g // 2, :],
                                      Bb[32 * g:32 * g + 1, 1:65, 1:G + 1])
                    nc.sync.dma_start(y1i_k[h:h + 64, g // 2, :],
                                      Bb[32 * g + 16:32 * g + 17, 1:65, 1:G + 1])
                ur_ps = ps2_pool.tile([N1, G], F32, tag="u")
                ui_ps = ps2_pool.tile([N1, G], F32, tag="u2")
                for c in range(2):
                    nc.tensor.matmul(ur_ps[:], cpt_bf[:, c, :], y1r_k[:, c, :],
                                     start=(c == 0), stop=False)
                    nc.tensor.matmul(ur_ps[:], spt_bf[:, c, :], y1i_k[:, c, :],
                                     start=False, stop=(c == 1))
                    nc.tensor.matmul(ui_ps[:], cpt_bf[:, c, :], y1i_k[:, c, :],
                                     start=(c == 0), stop=False)
                    nc.tensor.matmul(ui_ps[:], spnt_bf[:, c, :], y1r_k[:, c, :],
                                     start=False, stop=(c == 1))
                ur_sb = stg_pool.tile([N1, G], BF16, tag="ur")
                ui_sb = stg_pool.tile([N1, G], BF16, tag="ui")
                nc.vector.tensor_copy(ur_sb[:], ur_ps[:])
                nc.vector.tensor_copy(ui_sb[:], ui_ps[:])
                urt_sb = stg_pool.tile([128, 2, N1], BF16, tag="urt")
                uit_sb = stg_pool.tile([128, 2, N1], BF16, tag="uit")
                for c in range(2):
                    tp = ps2_pool.tile([128, N1], BF16, tag="u")
                    nc.tensor.transpose(tp[:], ur_sb[:, 128 * c:128 * (c + 1)], id_bf[0:N1, 0:N1])
                    nc.vector.tensor_copy(urt_sb[:, c, :], tp[:])
                    tp2 = ps2_pool.tile([128, N1], BF16, tag="u2")
                    nc.tensor.transpose(tp2[:], ui_sb[:, 128 * c:128 * (c + 1)], id_bf[0:N1, 0:N1])
                    nc.vector.tensor_copy(uit_sb[:, c, :], tp2[:])
                e_ps = ps2_pool.tile([N1, N1], F32, tag="u")
                for c in range(2):
                    nc.tensor.matmul(e_ps[:], urt_sb[:, c, :], cpt_bf[:, c, :],
                                     start=(c == 0), stop=False)
                    nc.tensor.matmul(e_ps[:], uit_sb[:, c, :], spt_bf[:, c, :],
                                     start=False, stop=(c == 1))
                e_sb = stg_pool.tile([N1, N1], F32, tag="e")
                nc.vector.tensor_copy(e_sb[:], e_ps[:])
                nc.sync.dma_start(eo[s], e_sb[:])

            # software pipeline: spectral(s) overlaps head(s+1)'s PE work;
            # head(s+1) issues first so its DVE copies aren't stuck behind
            # spectral's conv3-gated ops in the DVE FIFO.
            w_live = {0: head(0)}
            for s in range(ns):
                if s + 1 < ns:
                    w_live[s + 1] = head(s + 1)
                spectral(s)
                tail(s, w_live.pop(s))

    nc.compile()
    return nc


def _pack_lhsT(Wc):
    # Wc: [co, ci, 3, 3] complex -> [9, 32, 32] real lhsT blocks
    co, ci = Wc.shape[0], Wc.shape[1]
    out = np.zeros((9, 32, 32), np.float32)
    for dy in range(3):
        for dx in range(3):
            w = Wc[:, :, dy, dx]
            blk = out[3 * dy + dx]
            blk[0:ci, 0:co] = w.real.T
            blk[16:16 + ci, 0:co] = -w.imag.T
            blk[0:ci, 16:16 + co] = w.imag.T
            blk[16:16 + ci, 16:16 + co] = w.real.T
    return out


def _consts():
    m = np.arange(N1)
    k = np.arange(G)
    sst = (((-1.0) ** (m + 1))[:, None]
           * np.sin(2 * np.pi * np.outer(m + 1, k) / G)).astype(np.float32)
    Cp = (((-1.0) ** m)[:, None] * np.cos(2 * np.pi * np.outer(m, k) / G))
    Sp = (((-1.0) ** m)[:, None] * np.sin(2 * np.pi * np.outer(m, k) / G))
    # cpt[p, c, m] = Cp[m, c*128+p]
    cpt = np.ascontiguousarray(
        Cp.T.reshape(2, 128, N1).transpose(1, 0, 2)).astype(np.float32)
    spt = np.ascontiguousarray(
        Sp.T.reshape(2, 128, N1).transpose(1, 0, 2)).astype(np.float32)
    return {
        "sst": sst,
        "cpt": cpt,
        "spt": spt,
        "spnt": -spt,
        "ident": np.eye(128, dtype=np.float32),
    }


def make_core_inputs(r, w1r, w1i, w2r, w2i, w3r, w3i, wtr, wti, ns=NS):
    """Build one core's input map from its batch shard (leading dim ns)."""
    W1 = w1r + 1j * w1i
    W2 = w2r + 1j * w2i
    W3 = w3r + 1j * w3i

    def adj(W):
        return np.conj(np.swapaxes(np.swapaxes(W, 1, 2), -2, -1))

    A1, A2, A3 = adj(W1), adj(W2), adj(W3)
    wl = np.zeros((ns, 128, 6, 9, 32), np.float32)
    for s in range(ns):
        for l, W in enumerate((W1[s], W2[s], W3[s], A3[s], A2[s], A1[s])):
            blk = _pack_lhsT(W)  # [9, 32, 32]
            if l == 0:
                # layer-1 input is purely real (r_hat); its imag channels
                # hold stale data from the previous sample - never read them
                blk[:, 16:32, :] = 0.0
            wl[s, :, l] = np.tile(blk.transpose(1, 0, 2), (4, 1, 1))
    ta = np.empty((ns, 4, 32, 64, G), np.float32)
    tb = np.empty((ns, 4, 32, 64, G), np.float32)
    for g in range(4):
        rows = slice(64 * g, 64 * g + 64)
        ta[:, g, 0:16] = wtr[:, :, rows, :]
        tb[:, g, 0:16] = -wti[:, :, rows, :]
        ta[:, g, 16:32] = wtr[:, :, rows, :]
        tb[:, g, 16:32] = wti[:, :, rows, :]
    import ml_dtypes
    out = {
        "r4": np.ascontiguousarray(r[:, 0]).astype(np.float32),
        "wl": wl,
        "ta4": ta.reshape(ns, 128, 64, G).astype(ml_dtypes.bfloat16),
        "tb4": tb.reshape(ns, 128, 64, G).astype(ml_dtypes.bfloat16),
    }
    out.update(_consts())
    return out


_NC_CACHE = {}


def kernel(r, w1r, w1i, w2r, w2i, w3r, w3i, wtr, wti):
    B = r.shape[0]
    ns = B // NCORES
    if ns not in _NC_CACHE:
        _NC_CACHE[ns] = _build_nc(ns)
    nc = _NC_CACHE[ns]
    in_maps = []
    for c in range(NCORES):
        sl = slice(c * ns, (c + 1) * ns)
        in_maps.append(make_core_inputs(
            np.asarray(r[sl]), np.asarray(w1r[sl]), np.asarray(w1i[sl]),
            np.asarray(w2r[sl]), np.asarray(w2i[sl]), np.asarray(w3r[sl]),
            np.asarray(w3i[sl]), np.asarray(wtr[sl]), np.asarray(wti[sl]),
            ns=ns))
    res = run_bass_kernel_spmd(nc, in_maps, list(range(NCORES)))
    outs = [res.results[i]["eo"] for i in range(NCORES)]
    e = np.concatenate(outs, axis=0).reshape(B, 1, N1, N1).astype(np.float32)
    return e

